# revision 36
# baseline (speedup 1.0000x reference)
"""PALU low-rank Llama attention on 8 Trainium2 NeuronCores (v3).

Tensor-parallel over the 8 PALU groups (1 group = 4 heads per core); each core
computes its group's partial contribution to the full output (its slice of the
fused Wo contraction) and the host sums the 8 partials.

The kernel is TensorE-bound (cost-model engine busy ~1.16 ms of a ~1.29 ms
wall for the original version, confirmed by batched device timing), so v3
focuses on keeping the PE array dense and shrinking everything else:

  * The output projection (phase C) is emitted round-robin interleaved with
    attention (phase B) as soon as each head's attention output completes,
    so C matmuls fill B's exp-latency and PSUM-handover stalls.  B runs at
    single k-tile granularity, shrinking its PSUM footprint to 6 banks
    (2 score bufs + 4 output accumulators) and leaving 2 banks for C.
  * The V.T -> natural-layout transposes moved from TensorE (+PSUM +evac
    copies) to the DMA XBAR transpose path.
  * The output partial is written as fp16 (32 MB instead of 64 MB fp32).

Per-core pipeline otherwise follows the original design:
  A) projections, software-pipelined over 512-token chunks with a 4+4 PSUM
     bank split: pass1 accumulates K-latent (transposed) and V (natural) in
     banks 0-3 while pass2 accumulates Q directly in transposed (d, s)
     layout in banks 4-7.  K is reconstructed transposed from the latent via
     U.  RoPE is applied in the transposed layout using a signed
     half-rotation permutation matmul.  Q.T/K.T spill to DRAM scratch; V
     stays resident in SBUF.
  B) attention as one software-pipelined stream over every (head, q-chunk,
     k-tile) unit: scores computed transposed S.T = K.T' Q (k on partitions)
     at exact causal widths, exp on ScalarE straight out of PSUM (the
     diagonal mask is applied post-exp as a multiply by host-precomputed
     exp(mask)), then O = P.T' V with an extra all-ones V column producing
     the softmax denominators for free.  PV of unit n trails scores+exp of
     unit n+3 on TensorE so exp latency never stalls the PE.  O is
     normalized with per-partition reciprocal scalars and moved into O.T
     buffers via DMA XBAR transposes.
  C) fused output projection with the exact torch reshape/transpose
     semantics folded into strided access patterns on O.T.

All matmul operands are bf16 (TensorE full rate; fp8 DoubleRow was evaluated
and rejected: quantizing the O/Wo operands alone costs ~2.7e-2 absmax
relative error against the 2e-2 budget); PSUM accumulation is fp32.
"""

import numpy as np
import ml_dtypes

import concourse.bass as bass
import concourse.tile as tile
from concourse import bacc, mybir
from concourse.masks import make_identity
from concourse.bass_utils import run_bass_kernel_spmd

F32 = mybir.dt.float32
F16 = mybir.dt.float16
BF16 = mybir.dt.bfloat16
NPBF = ml_dtypes.bfloat16
EXP = mybir.ActivationFunctionType.Exp

B, S, HID = 2, 2048, 4096
NH, D = 32, 128
G, GS = 8, 4
RK, RV, FGD, GD = 256, 2048, 256, 512
NCORES = 8
CT = HID // 128        # 32 contraction tiles over hidden dim
NQC = S // 512         # 4 q-chunks of 512
THETA = 10000.0

_NC_CACHE = {}


def _install_loud_cc_hook():
    """Surface exceptions thrown inside the neuronx_cc compile hook (the C++
    callback boundary otherwise swallows them into an opaque INTERNAL error)."""
    if _NC_CACHE.get("loud_hook"):
        return
    import traceback
    from concourse import bass2jax
    orig = bass2jax.neuronx_cc_hook

    def loud_hook(*a, **kw):
        try:
            return orig(*a, **kw)
        except BaseException:
            traceback.print_exc()
            raise

    bass2jax.neuronx_cc_hook = loud_hook
    _NC_CACHE["loud_hook"] = True


def _copy(eng_nc, dst, src):
    """Engine-agnostic copy: ScalarE uses activation-Copy, others tensor_copy."""
    if hasattr(eng_nc, "tensor_copy"):
        eng_nc.tensor_copy(dst, src)
    else:
        eng_nc.copy(dst, src)


def _build_nc():
    """Build + compile the per-core Bass kernel (same NEFF for all cores)."""
    nc = bacc.Bacc(trn_type="TRN2", target_bir_lowering=False, debug=False)

    # host-prearranged inputs (see _host_prep): every bulk load is one 2D DMA
    hid_d = nc.dram_tensor("hid2", [B, NQC, 128, CT * 512], BF16,
                           kind="ExternalInput").ap()
    wq_d = nc.dram_tensor("wq2", [128, CT * GD], BF16, kind="ExternalInput").ap()
    wk_d = nc.dram_tensor("wk2", [128, CT * RK], BF16, kind="ExternalInput").ap()
    wv_d = nc.dram_tensor("wv2", [128, CT * FGD], BF16, kind="ExternalInput").ap()
    ug_d = nc.dram_tensor("ug2", [128, 2 * GD], BF16, kind="ExternalInput").ap()
    cosT_d = nc.dram_tensor("cosT", [B, 128, S], BF16, kind="ExternalInput").ap()
    sinT_d = nc.dram_tensor("sinT", [B, 128, S], BF16, kind="ExternalInput").ap()
    perm_d = nc.dram_tensor("perm", [128, 128], BF16, kind="ExternalInput").ap()
    msk_d = nc.dram_tensor("msk2", [B, 128, (S // 128) * 128], BF16,
                           kind="ExternalInput").ap()
    wo_d = nc.dram_tensor("wo2", [2, 128, 8 * 2048], BF16,
                          kind="ExternalInput").ap()
    out_d = nc.dram_tensor("out", [B, S, HID], F16, kind="ExternalOutput").ap()

    # DRAM scratch (per-core internal)
    qT_s = nc.dram_tensor("qT_s", [B, GS, 128, S], BF16).ap()
    kT_s = nc.dram_tensor("kT_s", [B, GS, 128, S], BF16).ap()

    with tile.TileContext(nc) as tc:
        # ---------- constants + tensors persisting across phases ----------
        with tc.tile_pool(name="const", bufs=1) as pc:
            perm_sb = pc.tile([128, 128], BF16, tag="perm")
            m_all = [pc.tile([128, (S // 128) * 128], BF16, name=f"mall{b}",
                             tag=f"mall{b}") for b in range(B)]
            v_hold = {}
            for b in range(B):
                for kt in range(S // 128):
                    v_hold[(b, kt)] = pc.tile(
                        [128, FGD + 2], BF16, name=f"vh{b}_{kt}",
                        tag=f"vh{b}_{kt}")

            # kt/qt reload pools opened around phase A so loads can be
            # emitted (and run) as soon as each batch's spills complete
            pkt = tc.alloc_tile_pool(name="ktb", bufs=4)
            pqt = tc.alloc_tile_pool(name="qtb", bufs=3)
            kt_tiles, qt_tiles = {}, {}
            qt_seq = [(i, qc) for i in range(B * GS) for qc in range(NQC)]

            def load_kt(i):
                bb, hl = divmod(i, GS)
                tkt = pkt.tile([128, S], BF16, tag="kt", name=f"kt{i}")
                nc.gpsimd.dma_start(tkt[:], kT_s[bb, hl])
                kt_tiles[i] = tkt

            def load_qt(pos):
                i, qc = qt_seq[pos]
                bb, hl = divmod(i, GS)
                tqt = pqt.tile([128, 512], BF16, tag="qt", name=f"qt{i}_{qc}")
                nc.sync.dma_start(tqt[:],
                                  qT_s[bb, hl, :, qc * 512:(qc + 1) * 512])
                qt_tiles[(i, qc)] = tqt

            # ================= Phase A: projections =================
            with tc.tile_pool(name="wts", bufs=1) as pw, \
                 tc.tile_pool(name="ht", bufs=2) as pht, \
                 tc.tile_pool(name="tab", bufs=1) as ptab, \
                 tc.tile_pool(name="stg", bufs=3) as pstg, \
                 tc.tile_pool(name="xnb", bufs=4) as pxn, \
                 tc.tile_pool(name="apsum", bufs=1, space="PSUM") as pps:

                # weight loads: block-DMAs spread across engine queues
                wq_sb = pw.tile([128, CT * GD], BF16, tag="wq")
                wk_sb = pw.tile([128, CT * RK], BF16, tag="wk")
                wv_sb = pw.tile([128, CT * FGD], BF16, tag="wv")
                ug_sb = pw.tile([128, 2 * GD], BF16, tag="ug")
                wblks = [(0, 2), (2, 8), (8, 16), (16, 24), (24, 32)]
                # wv rides the scalar HWDGE queue AHEAD of wq: pass1 needs
                # wk+wv just-in-time from the start, while wq (pass2) has
                # ~27us of slack - this unserializes the gpsimd queue at
                # kernel start
                for c0, c1 in wblks:
                    nc.scalar.dma_start(wv_sb[:, c0 * FGD:c1 * FGD],
                                        wv_d[:, c0 * FGD:c1 * FGD])
                    nc.gpsimd.dma_start(wk_sb[:, c0 * RK:c1 * RK],
                                        wk_d[:, c0 * RK:c1 * RK])
                for c0, c1 in wblks:
                    nc.scalar.dma_start(wq_sb[:, c0 * GD:c1 * GD],
                                        wq_d[:, c0 * GD:c1 * GD])
                nc.scalar.dma_start(ug_sb[:], ug_d[:])
                # const loads/memsets after the weight DMAs so they don't
                # delay the first pass1 matmuls on the gpsimd queue
                nc.gpsimd.dma_start(perm_sb[:], perm_d[:])
                for b in range(B):
                    nc.gpsimd.dma_start(m_all[b][:], msk_d[b])
                for key in v_hold:
                    nc.gpsimd.memset(v_hold[key][:, FGD:FGD + 2], 1.0)

                def rope_T(sb_h, rot_ps, cs, sn, dst_dram, eng):
                    """RoPE in transposed (d, s) layout.

                    sb_h: SBUF (128, 512) bf16 pre-rotation head tile.
                    rot_ps: PSUM (128, 512) f32 = perm @ sb_h (signed half
                    rotation).  out = sb_h * cs + rot_ps * sn -> bf16, spilled
                    to dst_dram."""
                    t1 = pstg.tile([128, 512], BF16, tag="t1")
                    t2 = pstg.tile([128, 512], BF16, tag="t2")
                    eng.tensor_mul(t1[:], sb_h[:], cs)
                    # GPSIMD cannot touch PSUM on hw: rot_ps read stays on DVE
                    nc.vector.tensor_mul(t2[:], rot_ps[:], sn)
                    xn = pxn.tile([128, 512], BF16, tag="xn")
                    eng.tensor_add(xn[:], t1[:], t2[:])
                    nc.sync.dma_start(dst_dram, xn[:])

                for b in range(B):
                    cosT_t = ptab.tile([128, S], BF16, tag="cos")
                    sinT_t = ptab.tile([128, S], BF16, tag="sin")
                    nc.scalar.dma_start(cosT_t[:], cosT_d[b])
                    nc.scalar.dma_start(sinT_t[:], sinT_d[b])
                    for qc in range(NQC):
                        # hidden chunk: 4 block DMAs into one big tile
                        t = pht.tile([128, CT * 512], BF16, tag="ht")
                        hblks = ([(0, 2), (2, 8)] if (b, qc) == (0, 0)
                                 else [(0, 8)]) + [(8, 16), (16, 24), (24, 32)]
                        for c0, c1 in hblks:
                            nc.sync.dma_start(
                                t[:, c0 * 512:c1 * 512],
                                hid_d[b, qc, :, c0 * 512:c1 * 512])
                        cs = cosT_t[:, qc * 512:(qc + 1) * 512]
                        sn = sinT_t[:, qc * 512:(qc + 1) * 512]

                        # ---- pass1: K-latent.T + V natural (PSUM banks 0-3)
                        klp = [pps.tile([128, 512], F32, tag="klp", bufs=2,
                                        name=f"klp{rt}") for rt in range(2)]
                        vp = [pps.tile([128, 512], F32, tag="vnp", bufs=2,
                                       name=f"vp{i}") for i in range(2)]
                        for ct in range(CT):
                            tt = t[:, ct * 512:(ct + 1) * 512]
                            st, sp = (ct == 0), (ct == CT - 1)
                            for rt in range(2):
                                nc.tensor.matmul(
                                    klp[rt][:],
                                    wk_sb[:, ct * RK + rt * 128:
                                          ct * RK + (rt + 1) * 128],
                                    tt, start=st, stop=sp)
                            for fp in range(2):
                                nc.tensor.matmul(
                                    vp[fp][:],
                                    wv_sb[:, ct * FGD + fp * 128:
                                          ct * FGD + (fp + 1) * 128],
                                    tt, start=st, stop=sp)

                        # ---- pass2: Q.T per head (PSUM banks 4-7)
                        qtp = [pps.tile([128, 512], F32, tag="qtp", bufs=4,
                                        name=f"qtp{h}") for h in range(GS)]
                        for ct in range(CT):
                            tt = t[:, ct * 512:(ct + 1) * 512]
                            st, sp = (ct == 0), (ct == CT - 1)
                            for h in range(GS):
                                nc.tensor.matmul(
                                    qtp[h][:],
                                    wq_sb[:, ct * GD + h * 128:
                                          ct * GD + (h + 1) * 128],
                                    tt, start=st, stop=sp)

                        # ---- evacuations (overlap pass2 on other engines)
                        klat_sb = []
                        for rt in range(2):
                            ksb = pstg.tile([128, 512], BF16, tag="klsb",
                                            name=f"ksb{rt}")
                            nc.scalar.copy(ksb[:], klp[rt][:])
                            klat_sb.append(ksb)
                        # V.T -> natural (q, f) directly via DMA XBAR
                        # transposes into the persistent v_hold tiles
                        for fp in range(2):
                            vtsb = pstg.tile([128, 512], BF16, tag="vtsb",
                                             name=f"vtsb{fp}")
                            eng = nc.scalar if fp else nc.vector
                            _copy(eng, vtsb[:], vp[fp][:])
                            for sub in range(4):
                                nc.sync.dma_start_transpose(
                                    v_hold[(b, qc * 4 + sub)][
                                        :, fp * 128:(fp + 1) * 128],
                                    vtsb[:, sub * 128:(sub + 1) * 128])

                        # ---- K.T reconstruction + rope (banks from klp tag)
                        k_sb = []
                        for h in range(GS):
                            kp = pps.tile([128, 512], F32, tag="klp", bufs=2,
                                          name=f"kTp{h}")
                            for rt in range(2):
                                nc.tensor.matmul(
                                    kp[:],
                                    ug_sb[:, rt * GD + h * 128:
                                          rt * GD + (h + 1) * 128],
                                    klat_sb[rt][:],
                                    start=(rt == 0), stop=(rt == 1))
                            ksb_h = pstg.tile([128, 512], BF16, tag="khsb")
                            nc.scalar.copy(ksb_h[:], kp[:])
                            k_sb.append(ksb_h)
                        # Q evacuations on DVE (parallel with scalar K evacs)
                        q_sb = []
                        for h in range(GS):
                            qsb_h = pstg.tile([128, 512], BF16, tag="qhsb")
                            if h % 2:
                                nc.scalar.copy(qsb_h[:], qtp[h][:])
                            else:
                                nc.vector.tensor_copy(qsb_h[:], qtp[h][:])
                            q_sb.append(qsb_h)
                        # signed half-rotation perm matmuls + rope + spill
                        for h in range(GS):
                            rot = pps.tile([128, 512], F32, tag="vnp", bufs=2,
                                           name=f"rotk{h}")
                            nc.tensor.matmul(rot[:], perm_sb[:], k_sb[h][:],
                                             start=True, stop=True)
                            rope_T(k_sb[h], rot, cs, sn,
                                   kT_s[b, h, :, qc * 512:(qc + 1) * 512],
                                   nc.gpsimd if h % 2 else nc.vector)
                        for h in range(GS):
                            rot = pps.tile([128, 512], F32, tag="vnp", bufs=2,
                                           name=f"rotq{h}")
                            nc.tensor.matmul(rot[:], perm_sb[:], q_sb[h][:],
                                             start=True, stop=True)
                            rope_T(q_sb[h], rot, cs, sn,
                                   qT_s[b, h, :, qc * 512:(qc + 1) * 512],
                                   nc.vector if h % 2 else nc.gpsimd)
                    if b == 0:
                        for hl in range(GS):
                            load_kt(hl)
                        load_qt(0)
                        load_qt(1)

            # ========== Phase B: attention, with phase C (output
            # projection) rounds interleaved round-robin ==========
            with tc.tile_pool(name="otb", bufs=6) as potb, \
                 tc.tile_pool(name="wob", bufs=1) as pwo, \
                 tc.tile_pool(name="cev", bufs=2) as pcev:
                # prefetch full Wo (bf16) while attention runs
                wo_t = []
                for half in range(2):
                    w = pwo.tile([128, 8 * 2048], BF16, name=f"wo{half}",
                                 tag=f"wo{half}")
                    for piece in range(2):
                        nc.gpsimd.dma_start(
                            w[:, piece * 8192:(piece + 1) * 8192],
                            wo_d[half][:, piece * 8192:(piece + 1) * 8192])
                    wo_t.append(w)

                import os as _os
                DEPTH = int(_os.environ.get("KV3_DEPTH", "3"))
                PUMP = int(_os.environ.get("KV3_PUMP", "2"))
                outq = {"sync": nc.sync, "gpsimd": nc.gpsimd,
                        "scalar": nc.scalar}[
                    _os.environ.get("KV3_OUTQ", "sync")]
                with tc.tile_pool(name="ptb", bufs=DEPTH + 2) as ppt, \
                     tc.tile_pool(name="bsm", bufs=DEPTH + 3) as pbs, \
                     tc.tile_pool(name="bpsum", bufs=1, space="PSUM") as bps:

                    # single stream over every (head, q-chunk, k-tile) unit;
                    # PV of unit n trails scores+exp of unit n+DEPTH on
                    # TensorE.  After each unit, up to two pending phase-C
                    # rounds (one PSUM accumulation each) are emitted to
                    # fill PE stalls.
                    units = []
                    for i in range(B * GS):
                        for qc in range(NQC):
                            for kt in range(4 * qc + 4):
                                units.append((i, qc, kt))

                    ob_map = {}
                    ot_map = {}
                    c_rounds = []

                    def emit_scores(i, qc, kt):
                        b, hl = divmod(i, GS)
                        kt_sb = kt_tiles[i]
                        qt_sb = qt_tiles[(i, qc)]
                        off = max(0, (kt - 4 * qc) * 128)
                        sc = bps.tile([128, 512], F32, tag="sc", bufs=2)
                        nc.tensor.matmul(
                            sc[:, off:512],
                            kt_sb[:, kt * 128:(kt + 1) * 128],
                            qt_sb[:, off:512],
                            start=True, stop=True)
                        pt = ppt.tile([128, 512], BF16, tag="pt")
                        nc.scalar.activation(pt[:, off:], sc[:, off:], EXP)
                        pd = None
                        if kt >= 4 * qc:
                            sub = kt - 4 * qc
                            pd = pbs.tile([128, 128], BF16, tag="ptd")
                            nc.vector.tensor_mul(
                                pd[:],
                                pt[:, sub * 128:(sub + 1) * 128],
                                m_all[b][:, kt * 128:(kt + 1) * 128])
                        return pt, pd

                    def emit_pv(i, qc, kt, pt, pd):
                        b, hl = divmod(i, GS)
                        ob = ob_map[(i, qc)]
                        for sub in range(4):
                            if kt <= 4 * qc + sub:
                                if kt == 4 * qc + sub:
                                    lhs = pd[:]
                                else:
                                    lhs = pt[:, sub * 128:(sub + 1) * 128]
                                nc.tensor.matmul(
                                    ob[sub][:], lhs, v_hold[(b, kt)][:],
                                    start=(kt == 0),
                                    stop=(kt == 4 * qc + sub))

                    def emit_evac(i, qc, sub):
                        b, hl = divmod(i, GS)
                        ob = ob_map[(i, qc)]
                        recip = pbs.tile([128, 1], F32, tag="rc")
                        nc.vector.reciprocal(recip[:], ob[sub][:, FGD:FGD + 1])
                        o_sb = pbs.tile([128, FGD], BF16, tag="osb")
                        nc.vector.tensor_scalar_mul(
                            o_sb[:], ob[sub][:, :FGD], recip[:])
                        col = qc * 512 + sub * 128
                        for fp in range(2):
                            nc.sync.dma_start_transpose(
                                ot_map[(i, fp)][:, col:col + 128],
                                o_sb[:, fp * 128:(fp + 1) * 128])
                        if qc == NQC - 1 and sub == 3:
                            bb, hl2 = divmod(i, GS)
                            for tt in range(hl2 * 4, hl2 * 4 + 4):
                                c_rounds.append((i, bb, tt, 0, None))

                    def emit_c_round(state, ev, i, b, tt):
                        """One output row-strip: [128 tokens, 4096] fp16.
                        Emitted as 8 single-bank accumulation rounds (state
                        counts rounds; each call emits one round of 8
                        matmuls + 1 evac)."""
                        half, mci = divmod(state, 4)
                        if ev is None:
                            ev = pcev.tile([128, HID], F16, tag="cev")
                        oc = bps.tile([128, 512], F32, tag="oc", bufs=2)
                        for j in range(4):
                            for fp in range(2):
                                otr = ot_map[(i, fp)][:].rearrange(
                                    "p (x s) -> p x s", s=4)
                                lhsT = otr[:, (tt % 4) * 128:
                                           (tt % 4 + 1) * 128, j]
                                rhs = wo_t[half][
                                    :, (j * 2 + fp) * 2048 + mci * 512:
                                    (j * 2 + fp) * 2048 + (mci + 1) * 512]
                                nc.tensor.matmul(
                                    oc[:], lhsT, rhs,
                                    start=(j == 0 and fp == 0),
                                    stop=(j == 3 and fp == 1))
                        seg = ev[:, half * 2048 + mci * 512:
                                 half * 2048 + (mci + 1) * 512]
                        if state % 2:
                            nc.scalar.copy(seg, oc[:])
                        else:
                            nc.vector.tensor_copy(seg, oc[:])
                        # write each finished half so the final DMA exposes
                        # only 4 KB/row instead of 8
                        if state == 3 or state == 7:
                            hh = state // 4
                            outq.dma_start(
                                out_d[b, tt * 128:(tt + 1) * 128,
                                      hh * 2048:(hh + 1) * 2048],
                                ev[:, hh * 2048:(hh + 1) * 2048])
                        if state == 7:
                            return None, None
                        return state + 1, ev

                    def pump_c(n):
                        for _ in range(n):
                            if not c_rounds:
                                return
                            i, b, tt, st, ev = c_rounds[0]
                            st, ev = emit_c_round(st, ev, i, b, tt)
                            if st is None:
                                c_rounds.pop(0)
                            else:
                                c_rounds[0] = (i, b, tt, st, ev)

                    from collections import deque
                    pending = deque()

                    def retire(unit):
                        i, qc, kt, pt, pd = unit
                        emit_pv(i, qc, kt, pt, pd)
                        if kt >= 4 * qc:
                            emit_evac(i, qc, kt - 4 * qc)
                        if kt == 4 * qc + 3:
                            ob_map.pop((i, qc))

                    for u, (i, qc, kt) in enumerate(units):
                        if qc == 0 and kt == 0:
                            if i < 4:
                                load_kt(i + 4)
                            for fp in range(2):
                                ot_map[(i, fp)] = potb.tile(
                                    [128, S], BF16, tag="ot",
                                    name=f"ot{i}_{fp}")
                        if kt == 0:
                            pos = i * NQC + qc
                            if pos + 2 < len(qt_seq):
                                load_qt(pos + 2)
                            ob_map[(i, qc)] = [
                                bps.tile([128, FGD + 2], F32, tag="ob",
                                         bufs=4, name=f"ob{k}")
                                for k in range(4)]
                        pending.append((i, qc, kt, *emit_scores(i, qc, kt)))
                        if len(pending) > DEPTH:
                            retire(pending.popleft())
                        pump_c(PUMP)
                    while pending:
                        retire(pending.popleft())
                    pump_c(10 ** 6)

            pqt.release()
            pkt.release()

    nc.compile()
    return nc


def _host_prep(inputs):
    """Slice/transposes per core; returns (in_maps, fallback_needed)."""
    hs = np.ascontiguousarray(inputs["hidden_states"], dtype=np.float32)
    mask = np.ascontiguousarray(inputs["attention_mask"], dtype=np.float32)
    pos = np.asarray(inputs["position_ids"])
    Wq = np.asarray(inputs["Wq"], dtype=np.float32)
    WVT = np.asarray(inputs["WVT"], dtype=np.float32)
    U = np.asarray(inputs["U"], dtype=np.float32)
    Wv = np.asarray(inputs["Wv"], dtype=np.float32)
    Wo = np.asarray(inputs["Wo"], dtype=np.float32)

    # Verify causal-family mask: strictly-lower 128-blocks all zero,
    # strictly-upper all <= -1e8 (else fall back to numpy reference).
    nt = S // 128
    mb = mask.reshape(B, nt, 128, nt, 128).transpose(0, 1, 3, 2, 4)
    lower_ok = True
    for b in range(B):
        for i in range(nt):
            for k in range(nt):
                blk = mb[b, i, k]
                if k < i and not np.all(blk == 0.0):
                    lower_ok = False
                if k > i and not np.all(blk <= -1e8):
                    lower_ok = False
    if not lower_ok:
        return None, True

    # hidden: (B, S, HID) -> (B, NQC, 128, CT*512) with
    # hid2[b, qc, p, ct*512+s] = hs[b, qc*512+s, ct*128+p]
    hid2 = np.ascontiguousarray(
        hs.reshape(B, NQC, 512, CT, 128).transpose(0, 1, 4, 3, 2)
        .reshape(B, NQC, 128, CT * 512)).astype(NPBF)

    # RoPE tables in transposed (d, s) layout; sign of the half-rotation is
    # folded into the permutation matrix.
    inv = 1.0 / (THETA ** (np.arange(0, D, 2, dtype=np.float32) / D))
    fr = pos.astype(np.float32)[..., None] * inv                # (B, S, 64)
    emb = np.concatenate([fr, fr], axis=-1)                     # (B, S, 128)
    cosT = np.ascontiguousarray(
        np.cos(emb).transpose(0, 2, 1)).astype(NPBF)            # (B, 128, S)
    sinT = np.ascontiguousarray(
        np.sin(emb).transpose(0, 2, 1)).astype(NPBF)
    # perm[k, m]: rot(x).T[m] = sum_k perm[k, m] * x.T[k]
    #   m <  64: rot[m] = -x[m+64]  -> perm[m+64, m] = -1
    #   m >= 64: rot[m] = +x[m-64]  -> perm[m-64, m] = +1
    perm = np.zeros((128, 128), np.float32)
    for m in range(64):
        perm[m + 64, m] = -1.0
        perm[m, m + 64] = 1.0
    perm = perm.astype(NPBF)

    # exp of transposed diagonal mask tiles (k, q), packed (B, 128, nt*128):
    # msk2[b, p, t*128+c] = exp(mask[b, 0, t*128+c, t*128+p]); applied as a
    # post-exp multiply on the diagonal probability subtiles.
    msk2 = np.empty((B, 128, nt * 128), np.float32)
    for b in range(B):
        for t in range(nt):
            msk2[b, :, t * 128:(t + 1) * 128] = np.exp(
                mask[b, 0, t * 128:(t + 1) * 128, t * 128:(t + 1) * 128].T)
    msk2 = np.ascontiguousarray(msk2).astype(NPBF)

    scale = np.float32(1.0 / np.sqrt(D))
    in_maps = []
    for g in range(NCORES):
        # weights laid out so SBUF tile cols match DRAM cols directly:
        # wq2[p, ct*GD+c] = (Wq_g.T * scale)[ct*128+p, c]
        wqT = (Wq[g * GD:(g + 1) * GD, :].T * scale)            # (HID, GD)
        wq2 = np.ascontiguousarray(
            wqT.reshape(CT, 128, GD).transpose(1, 0, 2)
            .reshape(128, CT * GD)).astype(NPBF)
        wkT = WVT[g * RK:(g + 1) * RK, :].T                     # (HID, RK)
        wk2 = np.ascontiguousarray(
            wkT.reshape(CT, 128, RK).transpose(1, 0, 2)
            .reshape(128, CT * RK)).astype(NPBF)
        wvT = Wv[g * RK:(g + 1) * RK, :].T                      # (HID, FGD)
        wv2 = np.ascontiguousarray(
            wvT.reshape(CT, 128, FGD).transpose(1, 0, 2)
            .reshape(128, CT * FGD)).astype(NPBF)
        ugT = U[:, g * RK:(g + 1) * RK].T                       # (RK, GD)
        ug2 = np.ascontiguousarray(
            ugT.reshape(2, 128, GD).transpose(1, 0, 2)
            .reshape(128, 2 * GD)).astype(NPBF)
        # wo2[half, p, (j*2+fp)*2048 + c] = Wo[half*2048+c, j*2048+g*FGD
        #                                       + fp*128 + p]
        wo2 = np.empty((2, 128, 8 * 2048), np.float32)
        for j in range(4):
            base = j * 2048 + g * FGD
            blk = Wo[:, base:base + FGD].T                      # (256, 4096)
            for half in range(2):
                for fp in range(2):
                    wo2[half, :, (j * 2 + fp) * 2048:(j * 2 + fp + 1) * 2048] \
                        = blk[fp * 128:(fp + 1) * 128,
                              half * 2048:(half + 1) * 2048]
        in_maps.append(dict(hid2=hid2, wq2=wq2, wk2=wk2, wv2=wv2, ug2=ug2,
                            cosT=cosT, sinT=sinT, perm=perm, msk2=msk2,
                            wo2=np.ascontiguousarray(wo2).astype(NPBF)))
    return in_maps, False


def _numpy_fallback(inputs):
    hs = np.asarray(inputs["hidden_states"], np.float32)
    mask = np.asarray(inputs["attention_mask"], np.float32)
    pos = np.asarray(inputs["position_ids"])
    Wq, WVT, U, Wv, Wo = (np.asarray(inputs[k], np.float32)
                          for k in ["Wq", "WVT", "U", "Wv", "Wo"])
    b, q = hs.shape[:2]
    qs = (hs @ Wq.T).reshape(b, q, NH, D).transpose(0, 2, 1, 3)
    klat = (hs @ WVT.T).reshape(b, q, G, RK).transpose(0, 2, 1, 3)
    vlat = (hs @ Wv.T).reshape(b, q, G, FGD).transpose(0, 2, 1, 3)
    Ugr = U.reshape(GD, G, RK)
    keys = np.einsum("bgsr,dgr->bgsd", klat, Ugr)
    keys = keys.transpose(0, 2, 1, 3).reshape(b, q, NH, D).transpose(0, 2, 1, 3)
    inv = 1.0 / (THETA ** (np.arange(0, D, 2, dtype=np.float32) / D))
    fr = pos.astype(np.float32)[..., None] * inv
    emb = np.concatenate([fr, fr], -1)
    cos, sin = np.cos(emb)[:, None], np.sin(emb)[:, None]

    def rot(x):
        return np.concatenate([-x[..., D // 2:], x[..., :D // 2]], -1)
    qs = qs * cos + rot(qs) * sin
    keys = keys * cos + rot(keys) * sin
    att = np.einsum("bhqd,bhkd->bhqk", qs, keys) / np.sqrt(D).astype(np.float32)
    att = att + mask
    att = att - att.max(-1, keepdims=True)
    att = np.exp(att)
    att = att / att.sum(-1, keepdims=True)
    aw = att.reshape(b, G, q * GS, q)
    o = np.einsum("bgik,bgkf->bgif", aw.astype(np.float32),
                  vlat.astype(np.float32))
    o = o.transpose(0, 2, 1, 3).reshape(b, q, 8192)
    return (o @ Wo.T).astype(np.float32)


def _make_timing_fn(nc):
    """Build the sharded jit callable for this Bass module.

    Mirrors bass2jax.run_bass_via_pjrt's multi-core path; returns
    (fn, in_names, out_names, out_avals, sharding)."""
    import jax
    from jax.sharding import Mesh, NamedSharding, PartitionSpec
    from jax.experimental.shard_map import shard_map
    from concourse import bass2jax, mybir as _mb

    bass2jax.install_neuronx_cc_hook()

    part_name = (nc.partition_id_tensor.name
                 if nc.partition_id_tensor is not None else None)
    in_names, out_names, out_avals = [], [], []
    for alloc in nc.m.functions[0].allocations:
        if not isinstance(alloc, _mb.MemoryLocationSet):
            continue
        name = alloc.memorylocations[0].name
        if alloc.kind == "ExternalInput":
            if name != part_name:
                in_names.append(name)
        elif alloc.kind == "ExternalOutput":
            out_names.append(name)
            out_avals.append(jax.core.ShapedArray(
                tuple(alloc.tensor_shape), _mb.dt.np(alloc.dtype)))
    n_params = len(in_names)
    all_names = in_names + out_names
    if part_name is not None:
        all_names = all_names + [part_name]

    def _body(*args):
        operands = list(args)
        if part_name is not None:
            operands.append(bass2jax.partition_id_tensor())
        outs = bass2jax._bass_exec_p.bind(
            *operands,
            out_avals=tuple(out_avals),
            in_names=tuple(all_names),
            out_names=tuple(out_names),
            lowering_input_output_aliases=(),
            sim_require_finite=True,
            sim_require_nnan=True,
            nc=nc,
        )
        return tuple(outs)

    devices = jax.devices()[:NCORES]
    mesh = Mesh(np.asarray(devices), ("core",))
    spec = PartitionSpec("core")
    n_outs = len(out_names)
    fn = jax.jit(
        shard_map(_body, mesh=mesh, in_specs=(spec,) * (n_params + n_outs),
                  out_specs=(spec,) * n_outs, check_rep=False),
        keep_unused=True,
    )
    return fn, in_names, out_names, out_avals, NamedSharding(mesh, spec)


def _run_spmd(nc, in_maps, time_iters=0):
    """Execute the SPMD kernel on the first NCORES neuron devices via PJRT."""
    import time as _time

    import jax

    if "timing_fn" not in _NC_CACHE:
        _NC_CACHE["timing_fn"] = _make_timing_fn(nc)
    fn, in_names, out_names, out_avals, sharding = _NC_CACHE["timing_fn"]
    dev_in = [
        jax.device_put(
            np.concatenate([np.asarray(m[name]) for m in in_maps], axis=0),
            sharding)
        for name in in_names
    ]
    dev_zero = [
        jax.device_put(
            np.zeros((NCORES * a.shape[0], *a.shape[1:]), a.dtype), sharding)
        for a in out_avals
    ]
    out = jax.block_until_ready(fn(*dev_in, *dev_zero))

    exec_ns = None
    if time_iters > 0:
        times = []
        for _ in range(time_iters):
            t0 = _time.perf_counter()
            r = jax.block_until_ready(fn(*dev_in, *dev_zero))
            times.append(_time.perf_counter() - t0)
        del r
        exec_ns = int(min(times) * 1e9)
        _NC_CACHE["bench_times"] = times

    results = []
    for c in range(NCORES):
        results.append({
            name: np.asarray(out[i]).reshape(NCORES, *out_avals[i].shape)[c]
            for i, name in enumerate(out_names)
        })
    return results, exec_ns


def kernel(**inputs):
    import os

    in_maps, fallback = _host_prep(inputs)
    if fallback:
        return _numpy_fallback(inputs)

    _install_loud_cc_hook()
    if "nc" not in _NC_CACHE:
        _NC_CACHE["nc"] = _build_nc()
    nc = _NC_CACHE["nc"]

    iters = int(os.environ.get("TRN_KERNEL_TIME_ITERS", "0"))
    results, exec_ns = _run_spmd(nc, in_maps, time_iters=iters)
    _NC_CACHE["last_exec_ns"] = exec_ns

    acc = np.zeros((B, S, HID), np.float64)
    for r in results:
        acc += r["out"].astype(np.float64)
    return acc.astype(np.float32)


# revision 43
# speedup vs baseline: 1.9194x; 1.9194x over previous
"""PALU low-rank Llama attention on 8 Trainium2 NeuronCores (v3).

Tensor-parallel over the 8 PALU groups (1 group = 4 heads per core); each core
computes its group's partial contribution to the full output (its slice of the
fused Wo contraction) and the host sums the 8 partials.

The kernel is TensorE-bound (cost-model engine busy ~1.16 ms of a ~1.29 ms
wall for the original version, confirmed by batched device timing), so v3
focuses on keeping the PE array dense and shrinking everything else:

  * The output projection (phase C) is emitted round-robin interleaved with
    attention (phase B) as soon as each head's attention output completes,
    so C matmuls fill B's exp-latency and PSUM-handover stalls.  B runs at
    single k-tile granularity, shrinking its PSUM footprint to 6 banks
    (2 score bufs + 4 output accumulators) and leaving 2 banks for C.
  * The V.T -> natural-layout transposes moved from TensorE (+PSUM +evac
    copies) to the DMA XBAR transpose path.
  * The output partial is written as fp16 (32 MB instead of 64 MB fp32).

Per-core pipeline otherwise follows the original design:
  A) projections, software-pipelined over 512-token chunks with a 4+4 PSUM
     bank split: pass1 accumulates K-latent (transposed) and V (natural) in
     banks 0-3 while pass2 accumulates Q directly in transposed (d, s)
     layout in banks 4-7.  K is reconstructed transposed from the latent via
     U.  RoPE is applied in the transposed layout using a signed
     half-rotation permutation matmul.  Q.T/K.T spill to DRAM scratch; V
     stays resident in SBUF.
  B) attention as one software-pipelined stream over every (head, q-chunk,
     k-tile) unit: scores computed transposed S.T = K.T' Q (k on partitions)
     at exact causal widths, exp on ScalarE straight out of PSUM (the
     diagonal mask is applied post-exp as a multiply by host-precomputed
     exp(mask)), then O = P.T' V with an extra all-ones V column producing
     the softmax denominators for free.  PV of unit n trails scores+exp of
     unit n+3 on TensorE so exp latency never stalls the PE.  O is
     normalized with per-partition reciprocal scalars and moved into O.T
     buffers via DMA XBAR transposes.
  C) fused output projection with the exact torch reshape/transpose
     semantics folded into strided access patterns on O.T.

All matmul operands are bf16 (TensorE full rate; fp8 DoubleRow was evaluated
and rejected: quantizing the O/Wo operands alone costs ~2.7e-2 absmax
relative error against the 2e-2 budget); PSUM accumulation is fp32.
"""

import numpy as np
import ml_dtypes

import concourse.bass as bass
import concourse.tile as tile
from concourse import bacc, mybir
from concourse.masks import make_identity
from concourse.bass_utils import run_bass_kernel_spmd

F32 = mybir.dt.float32
F16 = mybir.dt.float16
BF16 = mybir.dt.bfloat16
NPBF = ml_dtypes.bfloat16
EXP = mybir.ActivationFunctionType.Exp

B, S, HID = 2, 2048, 4096
NH, D = 32, 128
G, GS = 8, 4
RK, RV, FGD, GD = 256, 2048, 256, 512
NCORES = 8
CT = HID // 128        # 32 contraction tiles over hidden dim
NQC = S // 512         # 4 q-chunks of 512
THETA = 10000.0

_NC_CACHE = {}


def _install_loud_cc_hook():
    """Surface exceptions thrown inside the neuronx_cc compile hook (the C++
    callback boundary otherwise swallows them into an opaque INTERNAL error)."""
    if _NC_CACHE.get("loud_hook"):
        return
    import traceback
    from concourse import bass2jax
    orig = bass2jax.neuronx_cc_hook

    def loud_hook(*a, **kw):
        try:
            return orig(*a, **kw)
        except BaseException:
            traceback.print_exc()
            raise

    bass2jax.neuronx_cc_hook = loud_hook
    _NC_CACHE["loud_hook"] = True


def _copy(eng_nc, dst, src):
    """Engine-agnostic copy: ScalarE uses activation-Copy, others tensor_copy."""
    if hasattr(eng_nc, "tensor_copy"):
        eng_nc.tensor_copy(dst, src)
    else:
        eng_nc.copy(dst, src)


def _build_nc():
    """Build + compile the per-core Bass kernel (same NEFF for all cores)."""
    nc = bacc.Bacc(trn_type="TRN2", target_bir_lowering=False, debug=False)

    # host-prearranged inputs (see _host_prep): every bulk load is one 2D DMA
    hid_d = nc.dram_tensor("hid2", [B, NQC, 128, CT * 512], BF16,
                           kind="ExternalInput").ap()
    wq_d = nc.dram_tensor("wq2", [128, CT * GD], BF16, kind="ExternalInput").ap()
    wk_d = nc.dram_tensor("wk2", [128, CT * RK], BF16, kind="ExternalInput").ap()
    wv_d = nc.dram_tensor("wv2", [128, CT * FGD], BF16, kind="ExternalInput").ap()
    ug_d = nc.dram_tensor("ug2", [128, 2 * GD], BF16, kind="ExternalInput").ap()
    cosT_d = nc.dram_tensor("cosT", [B, 128, S], BF16, kind="ExternalInput").ap()
    sinT_d = nc.dram_tensor("sinT", [B, 128, S], BF16, kind="ExternalInput").ap()
    perm_d = nc.dram_tensor("perm", [128, 128], BF16, kind="ExternalInput").ap()
    msk_d = nc.dram_tensor("msk2", [B, 128, (S // 128) * 128], BF16,
                           kind="ExternalInput").ap()
    wo_d = nc.dram_tensor("wo2", [2, 128, 8 * 2048], BF16,
                          kind="ExternalInput").ap()
    out_d = nc.dram_tensor("out", [B, S, HID], F16, kind="ExternalOutput").ap()

    # DRAM scratch (per-core internal)
    qT_s = nc.dram_tensor("qT_s", [B, GS, 128, S], BF16).ap()
    kT_s = nc.dram_tensor("kT_s", [B, GS, 128, S], BF16).ap()

    with tile.TileContext(nc) as tc:
        # ---------- constants + tensors persisting across phases ----------
        with tc.tile_pool(name="const", bufs=1) as pc:
            perm_sb = pc.tile([128, 128], BF16, tag="perm")
            m_all = [pc.tile([128, (S // 128) * 128], BF16, name=f"mall{b}",
                             tag=f"mall{b}") for b in range(B)]
            v_hold = {}
            for b in range(B):
                for kt in range(S // 128):
                    v_hold[(b, kt)] = pc.tile(
                        [128, FGD + 2], BF16, name=f"vh{b}_{kt}",
                        tag=f"vh{b}_{kt}")

            # kt/qt reload pools opened around phase A so loads can be
            # emitted (and run) as soon as each batch's spills complete
            pkt = tc.alloc_tile_pool(name="ktb", bufs=4)
            pqt = tc.alloc_tile_pool(name="qtb", bufs=3)
            kt_tiles, qt_tiles = {}, {}
            qt_seq = [(i, qc) for i in range(B * GS) for qc in range(NQC)]

            def load_kt(i):
                bb, hl = divmod(i, GS)
                tkt = pkt.tile([128, S], BF16, tag="kt", name=f"kt{i}")
                nc.gpsimd.dma_start(tkt[:], kT_s[bb, hl])
                kt_tiles[i] = tkt

            def load_qt(pos):
                i, qc = qt_seq[pos]
                bb, hl = divmod(i, GS)
                tqt = pqt.tile([128, 512], BF16, tag="qt", name=f"qt{i}_{qc}")
                nc.sync.dma_start(tqt[:],
                                  qT_s[bb, hl, :, qc * 512:(qc + 1) * 512])
                qt_tiles[(i, qc)] = tqt

            # ================= Phase A: projections =================
            with tc.tile_pool(name="wts", bufs=1) as pw, \
                 tc.tile_pool(name="ht", bufs=2) as pht, \
                 tc.tile_pool(name="tab", bufs=1) as ptab, \
                 tc.tile_pool(name="stg", bufs=3) as pstg, \
                 tc.tile_pool(name="xnb", bufs=4) as pxn, \
                 tc.tile_pool(name="apsum", bufs=1, space="PSUM") as pps:

                # weight loads: block-DMAs spread across engine queues
                wq_sb = pw.tile([128, CT * GD], BF16, tag="wq")
                wk_sb = pw.tile([128, CT * RK], BF16, tag="wk")
                wv_sb = pw.tile([128, CT * FGD], BF16, tag="wv")
                ug_sb = pw.tile([128, 2 * GD], BF16, tag="ug")
                wblks = [(0, 2), (2, 8), (8, 16), (16, 24), (24, 32)]
                # wv rides the scalar HWDGE queue AHEAD of wq: pass1 needs
                # wk+wv just-in-time from the start, while wq (pass2) has
                # ~27us of slack - this unserializes the gpsimd queue at
                # kernel start
                for c0, c1 in wblks:
                    nc.scalar.dma_start(wv_sb[:, c0 * FGD:c1 * FGD],
                                        wv_d[:, c0 * FGD:c1 * FGD])
                    nc.gpsimd.dma_start(wk_sb[:, c0 * RK:c1 * RK],
                                        wk_d[:, c0 * RK:c1 * RK])
                for c0, c1 in wblks:
                    nc.scalar.dma_start(wq_sb[:, c0 * GD:c1 * GD],
                                        wq_d[:, c0 * GD:c1 * GD])
                nc.scalar.dma_start(ug_sb[:], ug_d[:])
                # const loads/memsets after the weight DMAs so they don't
                # delay the first pass1 matmuls on the gpsimd queue
                nc.gpsimd.dma_start(perm_sb[:], perm_d[:])
                for b in range(B):
                    nc.gpsimd.dma_start(m_all[b][:], msk_d[b])
                for key in v_hold:
                    nc.gpsimd.memset(v_hold[key][:, FGD:FGD + 2], 1.0)

                def rope_T(sb_h, rot_ps, cs, sn, dst_dram, eng):
                    """RoPE in transposed (d, s) layout.

                    sb_h: SBUF (128, 512) bf16 pre-rotation head tile.
                    rot_ps: PSUM (128, 512) f32 = perm @ sb_h (signed half
                    rotation).  out = sb_h * cs + rot_ps * sn -> bf16, spilled
                    to dst_dram."""
                    t1 = pstg.tile([128, 512], BF16, tag="t1")
                    t2 = pstg.tile([128, 512], BF16, tag="t2")
                    eng.tensor_mul(t1[:], sb_h[:], cs)
                    # GPSIMD cannot touch PSUM on hw: rot_ps read stays on DVE
                    nc.vector.tensor_mul(t2[:], rot_ps[:], sn)
                    xn = pxn.tile([128, 512], BF16, tag="xn")
                    eng.tensor_add(xn[:], t1[:], t2[:])
                    nc.sync.dma_start(dst_dram, xn[:])

                for b in range(B):
                    cosT_t = ptab.tile([128, S], BF16, tag="cos")
                    sinT_t = ptab.tile([128, S], BF16, tag="sin")
                    nc.scalar.dma_start(cosT_t[:], cosT_d[b])
                    nc.scalar.dma_start(sinT_t[:], sinT_d[b])
                    for qc in range(NQC):
                        # hidden chunk: 4 block DMAs into one big tile
                        t = pht.tile([128, CT * 512], BF16, tag="ht")
                        hblks = ([(0, 2), (2, 8)] if (b, qc) == (0, 0)
                                 else [(0, 8)]) + [(8, 16), (16, 24), (24, 32)]
                        for c0, c1 in hblks:
                            nc.sync.dma_start(
                                t[:, c0 * 512:c1 * 512],
                                hid_d[b, qc, :, c0 * 512:c1 * 512])
                        cs = cosT_t[:, qc * 512:(qc + 1) * 512]
                        sn = sinT_t[:, qc * 512:(qc + 1) * 512]

                        # ---- pass1: K-latent.T + V natural (PSUM banks 0-3)
                        klp = [pps.tile([128, 512], F32, tag="klp", bufs=2,
                                        name=f"klp{rt}") for rt in range(2)]
                        vp = [pps.tile([128, 512], F32, tag="vnp", bufs=2,
                                       name=f"vp{i}") for i in range(2)]
                        for ct in range(CT):
                            tt = t[:, ct * 512:(ct + 1) * 512]
                            st, sp = (ct == 0), (ct == CT - 1)
                            for rt in range(2):
                                nc.tensor.matmul(
                                    klp[rt][:],
                                    wk_sb[:, ct * RK + rt * 128:
                                          ct * RK + (rt + 1) * 128],
                                    tt, start=st, stop=sp)
                            for fp in range(2):
                                nc.tensor.matmul(
                                    vp[fp][:],
                                    wv_sb[:, ct * FGD + fp * 128:
                                          ct * FGD + (fp + 1) * 128],
                                    tt, start=st, stop=sp)

                        # ---- pass2: Q.T per head (PSUM banks 4-7)
                        qtp = [pps.tile([128, 512], F32, tag="qtp", bufs=4,
                                        name=f"qtp{h}") for h in range(GS)]
                        for ct in range(CT):
                            tt = t[:, ct * 512:(ct + 1) * 512]
                            st, sp = (ct == 0), (ct == CT - 1)
                            for h in range(GS):
                                nc.tensor.matmul(
                                    qtp[h][:],
                                    wq_sb[:, ct * GD + h * 128:
                                          ct * GD + (h + 1) * 128],
                                    tt, start=st, stop=sp)

                        # ---- evacuations (overlap pass2 on other engines)
                        klat_sb = []
                        for rt in range(2):
                            ksb = pstg.tile([128, 512], BF16, tag="klsb",
                                            name=f"ksb{rt}")
                            nc.scalar.copy(ksb[:], klp[rt][:])
                            klat_sb.append(ksb)
                        # V.T -> natural (q, f) directly via DMA XBAR
                        # transposes into the persistent v_hold tiles
                        for fp in range(2):
                            vtsb = pstg.tile([128, 512], BF16, tag="vtsb",
                                             name=f"vtsb{fp}")
                            eng = nc.scalar if fp else nc.vector
                            _copy(eng, vtsb[:], vp[fp][:])
                            for sub in range(4):
                                nc.sync.dma_start_transpose(
                                    v_hold[(b, qc * 4 + sub)][
                                        :, fp * 128:(fp + 1) * 128],
                                    vtsb[:, sub * 128:(sub + 1) * 128])

                        # ---- K.T reconstruction + rope (banks from klp tag)
                        k_sb = []
                        for h in range(GS):
                            kp = pps.tile([128, 512], F32, tag="klp", bufs=2,
                                          name=f"kTp{h}")
                            for rt in range(2):
                                nc.tensor.matmul(
                                    kp[:],
                                    ug_sb[:, rt * GD + h * 128:
                                          rt * GD + (h + 1) * 128],
                                    klat_sb[rt][:],
                                    start=(rt == 0), stop=(rt == 1))
                            ksb_h = pstg.tile([128, 512], BF16, tag="khsb")
                            nc.scalar.copy(ksb_h[:], kp[:])
                            k_sb.append(ksb_h)
                        # Q evacuations on DVE (parallel with scalar K evacs)
                        q_sb = []
                        for h in range(GS):
                            qsb_h = pstg.tile([128, 512], BF16, tag="qhsb")
                            if h % 2:
                                nc.scalar.copy(qsb_h[:], qtp[h][:])
                            else:
                                nc.vector.tensor_copy(qsb_h[:], qtp[h][:])
                            q_sb.append(qsb_h)
                        # signed half-rotation perm matmuls + rope + spill
                        for h in range(GS):
                            rot = pps.tile([128, 512], F32, tag="vnp", bufs=2,
                                           name=f"rotk{h}")
                            nc.tensor.matmul(rot[:], perm_sb[:], k_sb[h][:],
                                             start=True, stop=True)
                            rope_T(k_sb[h], rot, cs, sn,
                                   kT_s[b, h, :, qc * 512:(qc + 1) * 512],
                                   nc.gpsimd if h % 2 else nc.vector)
                        for h in range(GS):
                            rot = pps.tile([128, 512], F32, tag="vnp", bufs=2,
                                           name=f"rotq{h}")
                            nc.tensor.matmul(rot[:], perm_sb[:], q_sb[h][:],
                                             start=True, stop=True)
                            rope_T(q_sb[h], rot, cs, sn,
                                   qT_s[b, h, :, qc * 512:(qc + 1) * 512],
                                   nc.vector if h % 2 else nc.gpsimd)
                    if b == 0:
                        for hl in range(GS):
                            load_kt(hl)
                        load_qt(0)
                        load_qt(1)

            # ========== Phase B: attention, with phase C (output
            # projection) rounds interleaved round-robin ==========
            with tc.tile_pool(name="otb", bufs=6) as potb, \
                 tc.tile_pool(name="wob", bufs=1) as pwo, \
                 tc.tile_pool(name="cev", bufs=2) as pcev:
                # prefetch full Wo (bf16) while attention runs
                wo_t = []
                for half in range(2):
                    w = pwo.tile([128, 8 * 2048], BF16, name=f"wo{half}",
                                 tag=f"wo{half}")
                    for piece in range(2):
                        nc.gpsimd.dma_start(
                            w[:, piece * 8192:(piece + 1) * 8192],
                            wo_d[half][:, piece * 8192:(piece + 1) * 8192])
                    wo_t.append(w)

                import os as _os
                DEPTH = int(_os.environ.get("KV3_DEPTH", "3"))
                PUMP = int(_os.environ.get("KV3_PUMP", "2"))
                outq = {"sync": nc.sync, "gpsimd": nc.gpsimd,
                        "scalar": nc.scalar}[
                    _os.environ.get("KV3_OUTQ", "sync")]
                with tc.tile_pool(name="ptb", bufs=DEPTH + 2) as ppt, \
                     tc.tile_pool(name="bsm", bufs=DEPTH + 3) as pbs, \
                     tc.tile_pool(name="bpsum", bufs=1, space="PSUM") as bps:

                    # single stream over every (head, q-chunk, k-tile) unit;
                    # PV of unit n trails scores+exp of unit n+DEPTH on
                    # TensorE.  After each unit, up to two pending phase-C
                    # rounds (one PSUM accumulation each) are emitted to
                    # fill PE stalls.
                    units = []
                    for i in range(B * GS):
                        for qc in range(NQC):
                            for kt in range(4 * qc + 4):
                                units.append((i, qc, kt))

                    ob_map = {}
                    ot_map = {}
                    c_rounds = []

                    def emit_scores(i, qc, kt):
                        b, hl = divmod(i, GS)
                        kt_sb = kt_tiles[i]
                        qt_sb = qt_tiles[(i, qc)]
                        off = max(0, (kt - 4 * qc) * 128)
                        sc = bps.tile([128, 512], F32, tag="sc", bufs=2)
                        nc.tensor.matmul(
                            sc[:, off:512],
                            kt_sb[:, kt * 128:(kt + 1) * 128],
                            qt_sb[:, off:512],
                            start=True, stop=True)
                        pt = ppt.tile([128, 512], BF16, tag="pt")
                        nc.scalar.activation(pt[:, off:], sc[:, off:], EXP)
                        pd = None
                        if kt >= 4 * qc:
                            sub = kt - 4 * qc
                            pd = pbs.tile([128, 128], BF16, tag="ptd")
                            nc.vector.tensor_mul(
                                pd[:],
                                pt[:, sub * 128:(sub + 1) * 128],
                                m_all[b][:, kt * 128:(kt + 1) * 128])
                        return pt, pd

                    def emit_pv(i, qc, kt, pt, pd):
                        b, hl = divmod(i, GS)
                        ob = ob_map[(i, qc)]
                        for sub in range(4):
                            if kt <= 4 * qc + sub:
                                if kt == 4 * qc + sub:
                                    lhs = pd[:]
                                else:
                                    lhs = pt[:, sub * 128:(sub + 1) * 128]
                                nc.tensor.matmul(
                                    ob[sub][:], lhs, v_hold[(b, kt)][:],
                                    start=(kt == 0),
                                    stop=(kt == 4 * qc + sub))

                    def emit_evac(i, qc, sub):
                        b, hl = divmod(i, GS)
                        ob = ob_map[(i, qc)]
                        recip = pbs.tile([128, 1], F32, tag="rc")
                        nc.vector.reciprocal(recip[:], ob[sub][:, FGD:FGD + 1])
                        o_sb = pbs.tile([128, FGD], BF16, tag="osb")
                        nc.vector.tensor_scalar_mul(
                            o_sb[:], ob[sub][:, :FGD], recip[:])
                        col = qc * 512 + sub * 128
                        for fp in range(2):
                            nc.sync.dma_start_transpose(
                                ot_map[(i, fp)][:, col:col + 128],
                                o_sb[:, fp * 128:(fp + 1) * 128])
                        if qc == NQC - 1 and sub == 3:
                            bb, hl2 = divmod(i, GS)
                            for tt in range(hl2 * 4, hl2 * 4 + 4):
                                c_rounds.append((i, bb, tt, 0, None))

                    def emit_c_round(state, ev, i, b, tt):
                        """One output row-strip: [128 tokens, 4096] fp16.
                        Emitted as 8 single-bank accumulation rounds (state
                        counts rounds; each call emits one round of 8
                        matmuls + 1 evac)."""
                        half, mci = divmod(state, 4)
                        if ev is None:
                            ev = pcev.tile([128, HID], F16, tag="cev")
                        oc = bps.tile([128, 512], F32, tag="oc", bufs=2)
                        for j in range(4):
                            for fp in range(2):
                                otr = ot_map[(i, fp)][:].rearrange(
                                    "p (x s) -> p x s", s=4)
                                lhsT = otr[:, (tt % 4) * 128:
                                           (tt % 4 + 1) * 128, j]
                                rhs = wo_t[half][
                                    :, (j * 2 + fp) * 2048 + mci * 512:
                                    (j * 2 + fp) * 2048 + (mci + 1) * 512]
                                nc.tensor.matmul(
                                    oc[:], lhsT, rhs,
                                    start=(j == 0 and fp == 0),
                                    stop=(j == 3 and fp == 1))
                        seg = ev[:, half * 2048 + mci * 512:
                                 half * 2048 + (mci + 1) * 512]
                        if state % 2:
                            nc.scalar.copy(seg, oc[:])
                        else:
                            nc.vector.tensor_copy(seg, oc[:])
                        # write each finished half so the final DMA exposes
                        # only 4 KB/row instead of 8
                        if state == 3 or state == 7:
                            hh = state // 4
                            outq.dma_start(
                                out_d[b, tt * 128:(tt + 1) * 128,
                                      hh * 2048:(hh + 1) * 2048],
                                ev[:, hh * 2048:(hh + 1) * 2048])
                        if state == 7:
                            return None, None
                        return state + 1, ev

                    def pump_c(n):
                        for _ in range(n):
                            if not c_rounds:
                                return
                            i, b, tt, st, ev = c_rounds[0]
                            st, ev = emit_c_round(st, ev, i, b, tt)
                            if st is None:
                                c_rounds.pop(0)
                            else:
                                c_rounds[0] = (i, b, tt, st, ev)

                    from collections import deque
                    pending = deque()

                    def retire(unit):
                        i, qc, kt, pt, pd = unit
                        emit_pv(i, qc, kt, pt, pd)
                        if kt >= 4 * qc:
                            emit_evac(i, qc, kt - 4 * qc)
                        if kt == 4 * qc + 3:
                            ob_map.pop((i, qc))

                    for u, (i, qc, kt) in enumerate(units):
                        if qc == 0 and kt == 0:
                            if i < 4:
                                load_kt(i + 4)
                            for fp in range(2):
                                ot_map[(i, fp)] = potb.tile(
                                    [128, S], BF16, tag="ot",
                                    name=f"ot{i}_{fp}")
                        if kt == 0:
                            pos = i * NQC + qc
                            if pos + 2 < len(qt_seq):
                                load_qt(pos + 2)
                            ob_map[(i, qc)] = [
                                bps.tile([128, FGD + 2], F32, tag="ob",
                                         bufs=4, name=f"ob{k}")
                                for k in range(4)]
                        pending.append((i, qc, kt, *emit_scores(i, qc, kt)))
                        if len(pending) > DEPTH:
                            retire(pending.popleft())
                        pump_c(PUMP)
                    while pending:
                        retire(pending.popleft())
                    pump_c(10 ** 6)

            pqt.release()
            pkt.release()

    nc.compile()
    return nc


def _host_prep(inputs):
    """Slice/transposes per core; returns (in_maps, fallback_needed)."""
    hs = np.ascontiguousarray(inputs["hidden_states"], dtype=np.float32)
    mask = np.ascontiguousarray(inputs["attention_mask"], dtype=np.float32)
    pos = np.asarray(inputs["position_ids"])
    Wq = np.asarray(inputs["Wq"], dtype=np.float32)
    WVT = np.asarray(inputs["WVT"], dtype=np.float32)
    U = np.asarray(inputs["U"], dtype=np.float32)
    Wv = np.asarray(inputs["Wv"], dtype=np.float32)
    Wo = np.asarray(inputs["Wo"], dtype=np.float32)

    # Verify causal-family mask: strictly-lower 128-blocks all zero,
    # strictly-upper all <= -1e8 (else fall back to numpy reference).
    nt = S // 128
    mb = mask.reshape(B, nt, 128, nt, 128).transpose(0, 1, 3, 2, 4)
    lower_ok = True
    for b in range(B):
        for i in range(nt):
            for k in range(nt):
                blk = mb[b, i, k]
                if k < i and not np.all(blk == 0.0):
                    lower_ok = False
                if k > i and not np.all(blk <= -1e8):
                    lower_ok = False
    if not lower_ok:
        return None, True

    # hidden: (B, S, HID) -> (B, NQC, 128, CT*512) with
    # hid2[b, qc, p, ct*512+s] = hs[b, qc*512+s, ct*128+p]
    hid2 = np.ascontiguousarray(
        hs.reshape(B, NQC, 512, CT, 128).transpose(0, 1, 4, 3, 2)
        .reshape(B, NQC, 128, CT * 512)).astype(NPBF)

    # RoPE tables in transposed (d, s) layout; sign of the half-rotation is
    # folded into the permutation matrix.
    inv = 1.0 / (THETA ** (np.arange(0, D, 2, dtype=np.float32) / D))
    fr = pos.astype(np.float32)[..., None] * inv                # (B, S, 64)
    emb = np.concatenate([fr, fr], axis=-1)                     # (B, S, 128)
    cosT = np.ascontiguousarray(
        np.cos(emb).transpose(0, 2, 1)).astype(NPBF)            # (B, 128, S)
    sinT = np.ascontiguousarray(
        np.sin(emb).transpose(0, 2, 1)).astype(NPBF)
    # perm[k, m]: rot(x).T[m] = sum_k perm[k, m] * x.T[k]
    #   m <  64: rot[m] = -x[m+64]  -> perm[m+64, m] = -1
    #   m >= 64: rot[m] = +x[m-64]  -> perm[m-64, m] = +1
    perm = np.zeros((128, 128), np.float32)
    for m in range(64):
        perm[m + 64, m] = -1.0
        perm[m, m + 64] = 1.0
    perm = perm.astype(NPBF)

    # exp of transposed diagonal mask tiles (k, q), packed (B, 128, nt*128):
    # msk2[b, p, t*128+c] = exp(mask[b, 0, t*128+c, t*128+p]); applied as a
    # post-exp multiply on the diagonal probability subtiles.
    msk2 = np.empty((B, 128, nt * 128), np.float32)
    for b in range(B):
        for t in range(nt):
            msk2[b, :, t * 128:(t + 1) * 128] = np.exp(
                mask[b, 0, t * 128:(t + 1) * 128, t * 128:(t + 1) * 128].T)
    msk2 = np.ascontiguousarray(msk2).astype(NPBF)

    scale = np.float32(1.0 / np.sqrt(D))
    in_maps = []
    for g in range(NCORES):
        # weights laid out so SBUF tile cols match DRAM cols directly:
        # wq2[p, ct*GD+c] = (Wq_g.T * scale)[ct*128+p, c]
        wqT = (Wq[g * GD:(g + 1) * GD, :].T * scale)            # (HID, GD)
        wq2 = np.ascontiguousarray(
            wqT.reshape(CT, 128, GD).transpose(1, 0, 2)
            .reshape(128, CT * GD)).astype(NPBF)
        wkT = WVT[g * RK:(g + 1) * RK, :].T                     # (HID, RK)
        wk2 = np.ascontiguousarray(
            wkT.reshape(CT, 128, RK).transpose(1, 0, 2)
            .reshape(128, CT * RK)).astype(NPBF)
        wvT = Wv[g * RK:(g + 1) * RK, :].T                      # (HID, FGD)
        wv2 = np.ascontiguousarray(
            wvT.reshape(CT, 128, FGD).transpose(1, 0, 2)
            .reshape(128, CT * FGD)).astype(NPBF)
        ugT = U[:, g * RK:(g + 1) * RK].T                       # (RK, GD)
        ug2 = np.ascontiguousarray(
            ugT.reshape(2, 128, GD).transpose(1, 0, 2)
            .reshape(128, 2 * GD)).astype(NPBF)
        # wo2[half, p, (j*2+fp)*2048 + c] = Wo[half*2048+c, j*2048+g*FGD
        #                                       + fp*128 + p]
        wo2 = np.empty((2, 128, 8 * 2048), np.float32)
        for j in range(4):
            base = j * 2048 + g * FGD
            blk = Wo[:, base:base + FGD].T                      # (256, 4096)
            for half in range(2):
                for fp in range(2):
                    wo2[half, :, (j * 2 + fp) * 2048:(j * 2 + fp + 1) * 2048] \
                        = blk[fp * 128:(fp + 1) * 128,
                              half * 2048:(half + 1) * 2048]
        in_maps.append(dict(hid2=hid2, wq2=wq2, wk2=wk2, wv2=wv2, ug2=ug2,
                            cosT=cosT, sinT=sinT, perm=perm, msk2=msk2,
                            wo2=np.ascontiguousarray(wo2).astype(NPBF)))
    return in_maps, False


def _numpy_fallback(inputs):
    hs = np.asarray(inputs["hidden_states"], np.float32)
    mask = np.asarray(inputs["attention_mask"], np.float32)
    pos = np.asarray(inputs["position_ids"])
    Wq, WVT, U, Wv, Wo = (np.asarray(inputs[k], np.float32)
                          for k in ["Wq", "WVT", "U", "Wv", "Wo"])
    b, q = hs.shape[:2]
    qs = (hs @ Wq.T).reshape(b, q, NH, D).transpose(0, 2, 1, 3)
    klat = (hs @ WVT.T).reshape(b, q, G, RK).transpose(0, 2, 1, 3)
    vlat = (hs @ Wv.T).reshape(b, q, G, FGD).transpose(0, 2, 1, 3)
    Ugr = U.reshape(GD, G, RK)
    keys = np.einsum("bgsr,dgr->bgsd", klat, Ugr)
    keys = keys.transpose(0, 2, 1, 3).reshape(b, q, NH, D).transpose(0, 2, 1, 3)
    inv = 1.0 / (THETA ** (np.arange(0, D, 2, dtype=np.float32) / D))
    fr = pos.astype(np.float32)[..., None] * inv
    emb = np.concatenate([fr, fr], -1)
    cos, sin = np.cos(emb)[:, None], np.sin(emb)[:, None]

    def rot(x):
        return np.concatenate([-x[..., D // 2:], x[..., :D // 2]], -1)
    qs = qs * cos + rot(qs) * sin
    keys = keys * cos + rot(keys) * sin
    att = np.einsum("bhqd,bhkd->bhqk", qs, keys) / np.sqrt(D).astype(np.float32)
    att = att + mask
    att = att - att.max(-1, keepdims=True)
    att = np.exp(att)
    att = att / att.sum(-1, keepdims=True)
    aw = att.reshape(b, G, q * GS, q)
    o = np.einsum("bgik,bgkf->bgif", aw.astype(np.float32),
                  vlat.astype(np.float32))
    o = o.transpose(0, 2, 1, 3).reshape(b, q, 8192)
    return (o @ Wo.T).astype(np.float32)


def _make_timing_fn(nc):
    """Build the sharded jit callable for this Bass module.

    Mirrors bass2jax.run_bass_via_pjrt's multi-core path; returns
    (fn, in_names, out_names, out_avals, sharding)."""
    import jax
    from jax.sharding import Mesh, NamedSharding, PartitionSpec
    from jax.experimental.shard_map import shard_map
    from concourse import bass2jax, mybir as _mb

    bass2jax.install_neuronx_cc_hook()

    part_name = (nc.partition_id_tensor.name
                 if nc.partition_id_tensor is not None else None)
    in_names, out_names, out_avals = [], [], []
    for alloc in nc.m.functions[0].allocations:
        if not isinstance(alloc, _mb.MemoryLocationSet):
            continue
        name = alloc.memorylocations[0].name
        if alloc.kind == "ExternalInput":
            if name != part_name:
                in_names.append(name)
        elif alloc.kind == "ExternalOutput":
            out_names.append(name)
            out_avals.append(jax.core.ShapedArray(
                tuple(alloc.tensor_shape), _mb.dt.np(alloc.dtype)))
    n_params = len(in_names)
    all_names = in_names + out_names
    if part_name is not None:
        all_names = all_names + [part_name]

    def _body(*args):
        operands = list(args)
        if part_name is not None:
            operands.append(bass2jax.partition_id_tensor())
        outs = bass2jax._bass_exec_p.bind(
            *operands,
            out_avals=tuple(out_avals),
            in_names=tuple(all_names),
            out_names=tuple(out_names),
            lowering_input_output_aliases=(),
            sim_require_finite=True,
            sim_require_nnan=True,
            nc=nc,
        )
        return tuple(outs)

    devices = jax.devices()[:NCORES]
    mesh = Mesh(np.asarray(devices), ("core",))
    spec = PartitionSpec("core")
    n_outs = len(out_names)
    fn = jax.jit(
        shard_map(_body, mesh=mesh, in_specs=(spec,) * (n_params + n_outs),
                  out_specs=(spec,) * n_outs, check_rep=False),
        keep_unused=True,
    )
    return fn, in_names, out_names, out_avals, NamedSharding(mesh, spec)


def _run_spmd(nc, in_maps, time_iters=0):
    """Execute the SPMD kernel on the first NCORES neuron devices via PJRT."""
    import time as _time

    import jax

    if "timing_fn" not in _NC_CACHE:
        _NC_CACHE["timing_fn"] = _make_timing_fn(nc)
    fn, in_names, out_names, out_avals, sharding = _NC_CACHE["timing_fn"]
    dev_in = [
        jax.device_put(
            np.concatenate([np.asarray(m[name]) for m in in_maps], axis=0),
            sharding)
        for name in in_names
    ]
    dev_zero = [
        jax.device_put(
            np.zeros((NCORES * a.shape[0], *a.shape[1:]), a.dtype), sharding)
        for a in out_avals
    ]
    out = jax.block_until_ready(fn(*dev_in, *dev_zero))

    exec_ns = None
    if time_iters > 0:
        times = []
        for _ in range(time_iters):
            t0 = _time.perf_counter()
            r = jax.block_until_ready(fn(*dev_in, *dev_zero))
            times.append(_time.perf_counter() - t0)
        del r
        exec_ns = int(min(times) * 1e9)
        _NC_CACHE["bench_times"] = times

    results = []
    for c in range(NCORES):
        results.append({
            name: np.asarray(out[i]).reshape(NCORES, *out_avals[i].shape)[c]
            for i, name in enumerate(out_names)
        })
    return results, exec_ns


def kernel(**inputs):
    import os

    in_maps, fallback = _host_prep(inputs)
    if fallback:
        return _numpy_fallback(inputs)

    _install_loud_cc_hook()
    if "nc" not in _NC_CACHE:
        _NC_CACHE["nc"] = _build_nc()
    nc = _NC_CACHE["nc"]

    iters = int(os.environ.get("TRN_KERNEL_TIME_ITERS", "0"))
    results, exec_ns = _run_spmd(nc, in_maps, time_iters=iters)
    _NC_CACHE["last_exec_ns"] = exec_ns

    acc = np.zeros((B, S, HID), np.float64)
    for r in results:
        acc += r["out"].astype(np.float64)
    return acc.astype(np.float32)
